# revision 1
# baseline (speedup 1.0000x reference)
"""HeteroGNN (2-layer hetero GCN) Trainium2 kernel, 8-core SPMD.

Strategy: destination-sharded. Each core owns 6250 drug + 6250 dis nodes.
Feature tables (bf16 rows) live in per-core HBM; edge gathers use
dma_gather (custom SWDGE row gather); scatter-add is done as one-hot
"Msel" matmuls accumulating in PSUM (edges chunked 128 at a time, each
chunk's destinations confined to a 32-wide bin so PSUM offsets are
program constants shared by all cores). Layer-1 output slices are
exchanged with two AllGather collectives, then layer 2 + final linear.
All graph preprocessing (degrees, norms, chunking, padding to the
max-over-cores schedule) happens on host in numpy.
"""

import numpy as np
import ml_dtypes

import sys

for _p in ("/opt/trn_rl_repo",):
    if _p not in sys.path:
        sys.path.insert(0, _p)

import concourse.bass as bass
import concourse.mybir as mybir
from concourse import tile
from concourse.bass_utils import run_bass_kernel_spmd

BF16 = mybir.dt.bfloat16
F32 = mybir.dt.float32
I16 = mybir.dt.int16


class Cfg:
    def __init__(self, n=50000, e=800000, ncores=8, win=256, binw=32, group=2):
        self.N = n              # nodes per type
        self.E = e              # edges per relation
        self.NC = ncores
        self.S = n // ncores    # dst nodes per core per type
        self.WIN = win          # dsts per PSUM window
        self.BINW = binw        # dsts per bin (fixed psum offset granularity)
        self.GROUP = group      # windows per gather call
        self.NW = (self.S + win - 1) // win   # windows per type
        self.NG = (self.NW + group - 1) // group
        self.HALF = n // 2      # rows per gather half-table (int16 idx limit)
        assert self.HALF <= 32768
        self.D = 128
        self.OUT = 64

    def win_size(self, w):
        return min(self.WIN, self.S - w * self.WIN)

    def nbins(self, w):
        ws = self.win_size(w)
        return (ws + self.BINW - 1) // self.BINW


# relations per dst type: (reference rel index, src_is_dis)
# drug dst: rel 0 (dd, src drug), rel 3 (sd, src dis)
# dis  dst: rel 1 (ss, src dis),  rel 2 (ds, src drug)
REL_OF_T = {0: [(0, 0), (3, 1)], 1: [(1, 1), (2, 0)]}
SELF_LOOP = {0: True, 1: True, 2: False, 3: False}


def _prep_graph(cfg, edge_arrays):
    """edge_arrays: dict rel_idx -> (row, col) int64 full edge lists.
    Returns (meta, per_core) where meta is the SPMD-uniform schedule and
    per_core[c] = dict(idx=int16 [128, ICOLS], msel=f32 [128, MCOLS])."""
    N, S, WIN, BINW, NC = cfg.N, cfg.S, cfg.WIN, cfg.BINW, cfg.NC

    # chunks[(t, w, r, h, b)][core] = list of (idx128 array, dloc array, norm array)
    group_chunks = {}
    for t in (0, 1):
        for ri, (r, src_dis) in enumerate(REL_OF_T[t]):
            row, col = edge_arrays[r]
            if SELF_LOOP[r]:
                sl = np.arange(N, dtype=np.int64)
                row = np.concatenate([row, sl])
                col = np.concatenate([col, sl])
            deg_s = np.bincount(row, minlength=N).astype(np.float64)
            deg_d = np.bincount(col, minlength=N).astype(np.float64)
            norm = (deg_s[row] ** -0.5 * deg_d[col] ** -0.5).astype(np.float32)
            core = col // S
            d_loc = col % S
            w = d_loc // WIN
            b = (d_loc % WIN) // BINW
            h = row // cfg.HALF
            idx16 = (row % cfg.HALF).astype(np.int16)
            # group key: (core, w, b, h)
            nb_max = (WIN + BINW - 1) // BINW
            key = ((core * cfg.NW + w) * nb_max + b) * 2 + h
            order = np.argsort(key, kind="stable")
            key_s = key[order]
            uk, starts = np.unique(key_s, return_index=True)
            starts = list(starts) + [len(key_s)]
            for gi, k in enumerate(uk):
                sl_ = order[starts[gi]:starts[gi + 1]]
                kk = int(k)
                hh = kk % 2
                kk //= 2
                bb = kk % nb_max
                kk //= nb_max
                ww = kk % cfg.NW
                cc = kk // cfg.NW
                gkey = (t, ww, ri, hh, bb)
                group_chunks.setdefault(gkey, {c: [] for c in range(NC)})
                lst = group_chunks[gkey][cc]
                for s0 in range(0, len(sl_), 128):
                    ee = sl_[s0:s0 + 128]
                    lst.append((idx16[ee], (d_loc[ee] % WIN) % BINW, norm[ee]))

    # C_max per slot key
    cmax = {}
    for gkey, bycore in group_chunks.items():
        cmax[gkey] = max(len(v) for v in bycore.values())

    # Build uniform schedule.
    # calls: per (t, g, ri, h): list of chunk slot keys in order (w asc, b asc, dup)
    calls = []           # (t, g, ri, h, src_dis, idx_col_off, nchunks)
    call_lookup = {}     # (t, g, ri, h) -> call index
    icol = 0
    for t in (0, 1):
        for g in range(cfg.NG):
            for ri in range(2):
                src_dis = REL_OF_T[t][ri][1]
                for h in (0, 1):
                    nch = 0
                    for w in range(g * cfg.GROUP, min((g + 1) * cfg.GROUP, cfg.NW)):
                        for b in range(cfg.nbins(w)):
                            nch += cmax.get((t, w, ri, h, b), 0)
                    call_lookup[(t, g, ri, h)] = len(calls)
                    calls.append(dict(t=t, g=g, ri=ri, h=h, src_dis=src_dis,
                                      icol=icol, nchunks=nch))
                    icol += nch * 8
    ICOLS = max(icol, 8)

    # windows: per (t, w): msel col offset + chunk list
    windows = {}
    mcol = 0
    for t in (0, 1):
        for w in range(cfg.NW):
            wch = []   # (ri, h, j_in_call, psum_off)
            # j_in_call accumulators per (ri, h) for this group
            for ri in range(2):
                for h in (0, 1):
                    j = 0
                    g = w // cfg.GROUP
                    for w2 in range(g * cfg.GROUP, w):
                        for b in range(cfg.nbins(w2)):
                            j += cmax.get((t, w2, ri, h, b), 0)
                    for b in range(cfg.nbins(w)):
                        for d in range(cmax.get((t, w, ri, h, b), 0)):
                            wch.append((ri, h, j, b * BINW))
                            j += 1
            windows[(t, w)] = dict(mcol=mcol, chunks=wch)
            mcol += len(wch) * BINW
    MCOLS = max(mcol, BINW)

    meta = dict(calls=calls, call_lookup=call_lookup, windows=windows,
                ICOLS=ICOLS, MCOLS=MCOLS)

    # Per-core data arrays
    per_core = []
    for c in range(NC):
        idxa = np.zeros((16, ICOLS), np.int16)
        msel = np.zeros((128, MCOLS), np.float32)
        for call in calls:
            t, g, ri, h = call["t"], call["g"], call["ri"], call["h"]
            j = 0
            for w in range(g * cfg.GROUP, min((g + 1) * cfg.GROUP, cfg.NW)):
                for b in range(cfg.nbins(w)):
                    ck = group_chunks.get((t, w, ri, h, b))
                    lst = ck[c] if ck else []
                    for d in range(cmax.get((t, w, ri, h, b), 0)):
                        if d < len(lst):
                            ii, dd, nn = lst[d]
                            s0 = j * 128
                            sl = np.arange(s0, s0 + len(ii))
                            idxa[sl % 16, call["icol"] + sl // 16] = ii
                        j += 1
        # msel fill: walk windows
        for (t, w), wd in windows.items():
            dupc = {}
            for ci, (ri, h, jc, poff) in enumerate(wd["chunks"]):
                b = poff // BINW
                ck = group_chunks.get((t, w, ri, h, b))
                lst = ck[c] if ck else []
                d = dupc.get((ri, h, poff), 0)
                dupc[(ri, h, poff)] = d + 1
                if d < len(lst):
                    ii, dd, nn = lst[d]
                    m0 = wd["mcol"] + ci * BINW
                    msel[np.arange(len(ii)), m0 + dd] = nn
        idx_full = np.tile(idxa, (8, 1))
        per_core.append(dict(idx=idx_full,
                             msel=msel.astype(ml_dtypes.bfloat16)))
    return meta, per_core


def _build_program(cfg, meta):
    """Build the SPMD Bass program (same for all cores)."""
    from concourse import bacc

    NC, WIN, BINW = cfg.NC, cfg.WIN, cfg.BINW
    nc = bacc.Bacc("TRN2", target_bir_lowering=False, debug=False,
                   num_devices=NC)

    # I/O
    xt = {}
    for sd, nm in ((0, "d"), (1, "s")):
        for h in (0, 1):
            xt[(sd, h)] = nc.dram_tensor(
                f"x_{nm}_h{h}", [cfg.HALF, 128], BF16, kind="ExternalInput")
    idx_d = nc.dram_tensor("idx", [128, meta["ICOLS"]], I16, kind="ExternalInput")
    msel_d = nc.dram_tensor("msel", [128, meta["MCOLS"]], BF16, kind="ExternalInput")
    wts_d = nc.dram_tensor("wts", [2, 2, 2, 128, 128], BF16, kind="ExternalInput")
    linwt_d = nc.dram_tensor("linwt", [128, cfg.OUT], BF16, kind="ExternalInput")
    bias1_d = nc.dram_tensor("bias1", [2, 128, 2 * 128], F32, kind="ExternalInput")
    bias2_d = nc.dram_tensor("bias2", [2, 128, 1], F32, kind="ExternalInput")
    linb_d = nc.dram_tensor("linb", [128, 2 * cfg.OUT], F32, kind="ExternalInput")
    out_d = nc.dram_tensor("out", [2 * cfg.S, cfg.OUT], F32, kind="ExternalOutput")

    z_loc = [nc.dram_tensor(f"z_loc{t}", [cfg.S, 128], BF16) for t in (0, 1)]
    z_full = [nc.dram_tensor(f"z_full{t}", [cfg.N, 128], BF16,
                             addr_space="Shared") for t in (0, 1)]
    z_hi = [nc.dram_tensor(f"z_hi{t}", [cfg.HALF, 128], BF16) for t in (0, 1)]

    calls, windows = meta["calls"], meta["windows"]
    call_lookup = meta["call_lookup"]

    with tile.TileContext(nc) as tc:
        cpool = tc.alloc_tile_pool(name="const", bufs=1)
        gpool = tc.alloc_tile_pool(name="gather", bufs=2)
        ipool = tc.alloc_tile_pool(name="idx", bufs=2)
        mpool = tc.alloc_tile_pool(name="msel", bufs=2)
        apool = tc.alloc_tile_pool(name="aggs", bufs=2)
        zpool = tc.alloc_tile_pool(name="z", bufs=2)
        pagg = tc.alloc_tile_pool(name="pagg", bufs=2, space="PSUM")
        ptrf = tc.alloc_tile_pool(name="ptrf", bufs=2, space="PSUM")
        pfin = tc.alloc_tile_pool(name="pfin", bufs=2, space="PSUM")

        # constants to SBUF
        wsb = {}
        for l in (0, 1):
            for t in (0, 1):
                for ri in (0, 1):
                    w = cpool.tile([128, 128], BF16, tag=f"w{l}{t}{ri}", name=f"w_{l}{t}{ri}")
                    nc.sync.dma_start(w[:], wts_d[l, t, ri])
                    wsb[(l, t, ri)] = w
        linwt = cpool.tile([128, cfg.OUT], BF16, tag="linwt")
        nc.sync.dma_start(linwt[:], linwt_d[:])
        b1 = {}
        b2 = {}
        for t in (0, 1):
            b1[t] = cpool.tile([128, 256], F32, tag=f"b1{t}", name=f"b1_{t}")
            nc.sync.dma_start(b1[t][:], bias1_d[t])
            b2[t] = cpool.tile([128, 1], F32, tag=f"b2{t}", name=f"b2_{t}")
            nc.sync.dma_start(b2[t][:], bias2_d[t])
        linb = cpool.tile([128, 2 * cfg.OUT], F32, tag="linb")
        nc.sync.dma_start(linb[:], linb_d[:])

        def gather_tables(l, sd):
            if l == 0:
                return [xt[(sd, 0)].ap(), xt[(sd, 1)].ap()]
            return [z_full[sd][0:cfg.HALF, :], z_hi[sd].ap()]

        def do_layer(l):
            gtiles = {}
            for t in (0, 1):
                for w in range(cfg.NW):
                    ws = cfg.win_size(w)
                    g = w // cfg.GROUP
                    if w % cfg.GROUP == 0:
                        for ri in range(2):
                            for h in (0, 1):
                                call = calls[call_lookup[(t, g, ri, h)]]
                                C = call["nchunks"]
                                if C == 0:
                                    gtiles[(ri, h)] = None
                                    continue
                                it = ipool.tile([128, C * 8], I16, tag=f"i{ri}{h}", name=f"it{ri}{h}")
                                nc.sync.dma_start(
                                    it[:], idx_d[:, call["icol"]:call["icol"] + C * 8])
                                gt = gpool.tile([128, C, 128], BF16, tag=f"g{ri}{h}", name=f"gt{ri}{h}")
                                src = gather_tables(l, call["src_dis"])[h]
                                for j0 in range(0, C, 8):
                                    cs = min(8, C - j0)
                                    nc.gpsimd.dma_gather(
                                        gt[:, j0:j0 + cs, :], src,
                                        it[:, j0 * 8:(j0 + cs) * 8],
                                        cs * 128, cs * 128, 128)
                                gtiles[(ri, h)] = gt
                    wd = windows[(t, w)]
                    nch = len(wd["chunks"])
                    if nch:
                        mt = mpool.tile([128, nch * BINW], BF16, tag="m", name="mt")
                        nc.sync.dma_start(
                            mt[:], msel_d[:, wd["mcol"]:wd["mcol"] + nch * BINW])
                    aggP = [pagg.tile([128, 256], F32, tag=f"agg{r}", name=f"aggP{r}") for r in (0, 1)]
                    nc.vector.memset(aggP[0][:], 0.0)
                    nc.vector.memset(aggP[1][:], 0.0)
                    last_of_r = {}
                    for ci, (ri, h, j, poff) in enumerate(wd["chunks"]):
                        last_of_r[ri] = ci
                    for ci, (ri, h, j, poff) in enumerate(wd["chunks"]):
                        gt = gtiles[(ri, h)]
                        nc.tensor.matmul(
                            aggP[ri][:, poff:poff + BINW],
                            gt[:, j, :],
                            mt[:, ci * BINW:(ci + 1) * BINW],
                            start=False, stop=(last_of_r[ri] == ci),
                            skip_group_check=True)
                    aggS = []
                    for r in (0, 1):
                        a = apool.tile([128, 256], BF16, tag=f"as{r}", name=f"aggS{r}")
                        nc.vector.tensor_copy(a[:, 0:ws], aggP[r][:, 0:ws])
                        aggS.append(a)
                    if l == 0:
                        outP = ptrf.tile([128, 256], F32, tag="tp")
                        nsub = (ws + 127) // 128
                        for j in range(nsub):
                            m = min(128, ws - j * 128)
                            nc.tensor.matmul(
                                outP[0:m, j * 128:j * 128 + 128],
                                aggS[0][:, j * 128:j * 128 + m],
                                wsb[(l, t, 0)][:], start=True, stop=False)
                            nc.tensor.matmul(
                                outP[0:m, j * 128:j * 128 + 128],
                                aggS[1][:, j * 128:j * 128 + m],
                                wsb[(l, t, 1)][:], start=False, stop=True)
                        tmp = zpool.tile([128, 256], F32, tag="tmp")
                        zsb = zpool.tile([128, 256], BF16, tag="zsb")
                        for j in range(nsub):
                            m = min(128, ws - j * 128)
                            sl = slice(j * 128, j * 128 + 128)
                            nc.vector.tensor_add(tmp[0:m, sl], outP[0:m, sl],
                                                 b1[t][0:m, sl])
                            nc.scalar.activation(
                                zsb[0:m, sl], tmp[0:m, sl],
                                mybir.ActivationFunctionType.Relu)
                            base = w * WIN + j * 128
                            nc.sync.dma_start(
                                z_loc[t][base:base + m, :], zsb[0:m, sl])
                    else:
                        z2P = ptrf.tile([128, 256], F32, tag="tp")
                        nc.tensor.matmul(z2P[:, 0:ws], wsb[(l, t, 0)][:],
                                         aggS[0][:, 0:ws], start=True, stop=False)
                        nc.tensor.matmul(z2P[:, 0:ws], wsb[(l, t, 1)][:],
                                         aggS[1][:, 0:ws], start=False, stop=True)
                        z2T = zpool.tile([128, 256], BF16, tag="z2t")
                        nc.scalar.activation(z2T[:, 0:ws], z2P[:, 0:ws],
                                             mybir.ActivationFunctionType.Relu,
                                             bias=b2[t][:])
                        fP = pfin.tile([128, 128], F32, tag="fp")
                        nsub = (ws + 127) // 128
                        for j in range(nsub):
                            m = min(128, ws - j * 128)
                            nc.tensor.matmul(
                                fP[0:m, j * 64:j * 64 + 64],
                                z2T[:, j * 128:j * 128 + m],
                                linwt[:], start=True, stop=True)
                        fo = zpool.tile([128, 128], F32, tag="fo")
                        for j in range(nsub):
                            m = min(128, ws - j * 128)
                            sl = slice(j * 64, j * 64 + 64)
                            nc.vector.tensor_add(fo[0:m, sl], fP[0:m, sl],
                                                 linb[0:m, sl])
                            base = t * cfg.S + w * WIN + j * 128
                            nc.sync.dma_start(out_d[base:base + m, :], fo[0:m, sl])

        stage = getattr(cfg, "stage", 2)
        do_layer(0)
        for t in (0, 1):
            if stage < 1:
                break
            nc.gpsimd.collective_compute(
                "AllGather", mybir.AluOpType.bypass,
                replica_groups=[list(range(NC))],
                ins=[z_loc[t].ap().opt()], outs=[z_full[t].ap().opt()])
        if stage >= 1:
            tc.strict_bb_all_engine_barrier()
            for t in (0, 1):
                nc.sync.dma_start(z_hi[t].ap(), z_full[t][cfg.HALF:2 * cfg.HALF, :])
        if stage >= 2:
            do_layer(1)

        for p in (pfin, ptrf, pagg, zpool, apool, mpool, ipool, gpool, cpool):
            p.release()

    nc.compile()
    return nc


def _make_inputs(cfg, per_core, x_drug, x_dis, Ws, bs, lin_w, lin_b):
    bf = ml_dtypes.bfloat16
    xb = {0: x_drug.astype(bf), 1: x_dis.astype(bf)}
    wts = np.zeros((2, 2, 2, 128, 128), np.float32)
    b1 = np.zeros((2, 128, 256), np.float32)
    b2 = np.zeros((2, 128, 1), np.float32)
    for l in (0, 1):
        for t in (0, 1):
            for ri in (0, 1):
                r = REL_OF_T[t][ri][0]
                wts[l, t, ri] = Ws[l, r]
            bsum = bs[l, REL_OF_T[t][0][0]] + bs[l, REL_OF_T[t][1][0]]
            if l == 0:
                b1[t] = np.tile(bsum[None, :], (128, 2))
            else:
                b2[t] = bsum[:, None].astype(np.float32)
    shared = {
        "wts": wts.astype(bf),
        "linwt": lin_w.T.astype(bf).copy(),
        "bias1": b1,
        "bias2": b2,
        "linb": np.tile(lin_b[None, :], (128, 2)).astype(np.float32),
    }
    for sd, nm in ((0, "d"), (1, "s")):
        for h in (0, 1):
            shared[f"x_{nm}_h{h}"] = np.ascontiguousarray(
                xb[sd][h * cfg.HALF:(h + 1) * cfg.HALF])
    in_maps = []
    for c in range(cfg.NC):
        m = dict(shared)
        m["idx"] = per_core[c]["idx"]
        m["msel"] = per_core[c]["msel"]
        in_maps.append(m)
    return in_maps


def run(cfg, x_drug, x_dis, eis, Ws, bs, lin_w, lin_b, trace=False):
    edge_arrays = {r: (eis[r][0].astype(np.int64), eis[r][1].astype(np.int64))
                   for r in range(4)}
    meta, per_core = _prep_graph(cfg, edge_arrays)
    nc = _build_program(cfg, meta)
    in_maps = _make_inputs(cfg, per_core, x_drug, x_dis, Ws, bs, lin_w, lin_b)
    res = run_bass_kernel_spmd(nc, in_maps, core_ids=list(range(cfg.NC)),
                               trace=trace)
    drug = np.zeros((cfg.N, cfg.OUT), np.float32)
    dis = np.zeros((cfg.N, cfg.OUT), np.float32)
    for c in range(cfg.NC):
        o = res.results[c]["out"]
        drug[c * cfg.S:(c + 1) * cfg.S] = o[:cfg.S]
        dis[c * cfg.S:(c + 1) * cfg.S] = o[cfg.S:]
    return (drug, dis), res


def kernel(x_drug, x_dis, ei_dd, ei_ss, ei_ds, ei_sd, Ws, bs, lin_w, lin_b):
    cfg = Cfg()
    eis = {0: np.asarray(ei_dd), 1: np.asarray(ei_ss),
           2: np.asarray(ei_ds), 3: np.asarray(ei_sd)}
    out, _ = run(cfg, np.asarray(x_drug), np.asarray(x_dis), eis,
                 np.asarray(Ws), np.asarray(bs),
                 np.asarray(lin_w), np.asarray(lin_b))
    return out



# revision 8
# speedup vs baseline: 2.0813x; 2.0813x over previous
"""HeteroGNN (2-layer hetero GCN) Trainium2 kernel, 8-core SPMD.

Strategy: destination-sharded. Each core owns 6250 drug + 6250 dis nodes.
Feature tables (bf16 rows) live in per-core HBM; edge gathers use
dma_gather (custom SWDGE row gather); scatter-add is done as one-hot
"Msel" matmuls accumulating in PSUM (edges chunked 128 at a time, each
chunk's destinations confined to a 32-wide bin so PSUM offsets are
program constants shared by all cores). Layer-1 output slices are
exchanged with two AllGather collectives, then layer 2 + final linear.
All graph preprocessing (degrees, norms, chunking, padding to the
max-over-cores schedule) happens on host in numpy.
"""

import numpy as np
import ml_dtypes

import sys

for _p in ("/opt/trn_rl_repo",):
    if _p not in sys.path:
        sys.path.insert(0, _p)

import concourse.bass as bass
import concourse.mybir as mybir
from concourse import tile
from concourse.bass_utils import run_bass_kernel_spmd

BF16 = mybir.dt.bfloat16
F32 = mybir.dt.float32
I16 = mybir.dt.int16


class Cfg:
    def __init__(self, n=50000, e=800000, ncores=8, win=256, binw=32, group=2):
        self.N = n              # nodes per type
        self.E = e              # edges per relation
        self.NC = ncores
        self.S = n // ncores    # dst nodes per core per type
        self.WIN = win          # dsts per PSUM window
        self.BINW = binw        # dsts per bin (fixed psum offset granularity)
        self.GROUP = group      # windows per gather call
        self.NW = (self.S + win - 1) // win   # windows per type
        self.NG = (self.NW + group - 1) // group
        self.HALF = n // 2      # rows per gather half-table (int16 idx limit)
        assert self.HALF <= 32768
        self.D = 128
        self.OUT = 64

    def win_size(self, w):
        return min(self.WIN, self.S - w * self.WIN)

    def nbins(self, w):
        ws = self.win_size(w)
        return (ws + self.BINW - 1) // self.BINW


# relations per dst type: (reference rel index, src_is_dis)
# drug dst: rel 0 (dd, src drug), rel 3 (sd, src dis)
# dis  dst: rel 1 (ss, src dis),  rel 2 (ds, src drug)
REL_OF_T = {0: [(0, 0), (3, 1)], 1: [(1, 1), (2, 0)]}
SELF_LOOP = {0: True, 1: True, 2: False, 3: False}


def _prep_graph(cfg, edge_arrays):
    """edge_arrays: dict rel_idx -> (row, col) int64 full edge lists.
    Returns (meta, per_core) where meta is the SPMD-uniform schedule and
    per_core[c] = dict(idx=int16 [128, ICOLS], msel=f32 [128, MCOLS])."""
    N, S, WIN, BINW, NC = cfg.N, cfg.S, cfg.WIN, cfg.BINW, cfg.NC

    # chunks[(t, w, r, h, b)][core] = list of (idx128 array, dloc array, norm array)
    group_chunks = {}
    for t in (0, 1):
        for ri, (r, src_dis) in enumerate(REL_OF_T[t]):
            row, col = edge_arrays[r]
            if SELF_LOOP[r]:
                sl = np.arange(N, dtype=np.int64)
                row = np.concatenate([row, sl])
                col = np.concatenate([col, sl])
            deg_s = np.bincount(row, minlength=N).astype(np.float64)
            deg_d = np.bincount(col, minlength=N).astype(np.float64)
            norm = (deg_s[row] ** -0.5 * deg_d[col] ** -0.5).astype(np.float32)
            core = col // S
            d_loc = col % S
            w = d_loc // WIN
            b = (d_loc % WIN) // BINW
            h = row // cfg.HALF
            idx16 = (row % cfg.HALF).astype(np.int16)
            # group key: (core, w, b, h)
            nb_max = (WIN + BINW - 1) // BINW
            key = ((core * cfg.NW + w) * nb_max + b) * 2 + h
            order = np.argsort(key, kind="stable")
            key_s = key[order]
            uk, starts = np.unique(key_s, return_index=True)
            starts = list(starts) + [len(key_s)]
            for gi, k in enumerate(uk):
                sl_ = order[starts[gi]:starts[gi + 1]]
                kk = int(k)
                hh = kk % 2
                kk //= 2
                bb = kk % nb_max
                kk //= nb_max
                ww = kk % cfg.NW
                cc = kk // cfg.NW
                gkey = (t, ww, ri, hh, bb)
                group_chunks.setdefault(gkey, {c: [] for c in range(NC)})
                lst = group_chunks[gkey][cc]
                for s0 in range(0, len(sl_), 128):
                    ee = sl_[s0:s0 + 128]
                    lst.append((idx16[ee], (d_loc[ee] % WIN) % BINW, norm[ee]))

    # C_max per slot key
    cmax = {}
    for gkey, bycore in group_chunks.items():
        cmax[gkey] = max(len(v) for v in bycore.values())

    # Build uniform schedule.
    # calls: per (t, g, ri, h): list of chunk slot keys in order (w asc, b asc, dup)
    calls = []           # (t, g, ri, h, src_dis, idx_col_off, nchunks)
    call_lookup = {}     # (t, g, ri, h) -> call index
    icol = 0
    for t in (0, 1):
        for g in range(cfg.NG):
            for ri in range(2):
                src_dis = REL_OF_T[t][ri][1]
                for h in (0, 1):
                    nch = 0
                    for w in range(g * cfg.GROUP, min((g + 1) * cfg.GROUP, cfg.NW)):
                        for b in range(cfg.nbins(w)):
                            nch += cmax.get((t, w, ri, h, b), 0)
                    call_lookup[(t, g, ri, h)] = len(calls)
                    calls.append(dict(t=t, g=g, ri=ri, h=h, src_dis=src_dis,
                                      icol=icol, nchunks=nch))
                    icol += nch * 8
    ICOLS = max(icol, 8)

    # windows: per (t, w): msel col offset + chunk list
    windows = {}
    mcol = 0
    for t in (0, 1):
        for w in range(cfg.NW):
            wch = []   # (ri, h, j_in_call, psum_off)
            # j_in_call accumulators per (ri, h) for this group
            for ri in range(2):
                for h in (0, 1):
                    j = 0
                    g = w // cfg.GROUP
                    for w2 in range(g * cfg.GROUP, w):
                        for b in range(cfg.nbins(w2)):
                            j += cmax.get((t, w2, ri, h, b), 0)
                    for b in range(cfg.nbins(w)):
                        for d in range(cmax.get((t, w, ri, h, b), 0)):
                            wch.append((ri, h, j, b * BINW))
                            j += 1
            windows[(t, w)] = dict(mcol=mcol, chunks=wch)
            mcol += len(wch) * BINW
    MCOLS = max(mcol, BINW)

    meta = dict(calls=calls, call_lookup=call_lookup, windows=windows,
                ICOLS=ICOLS, MCOLS=MCOLS)

    # Per-core data arrays
    per_core = []
    for c in range(NC):
        idxa = np.zeros((16, ICOLS), np.int16)
        msel = np.zeros((128, MCOLS), np.float32)
        for call in calls:
            t, g, ri, h = call["t"], call["g"], call["ri"], call["h"]
            j = 0
            for w in range(g * cfg.GROUP, min((g + 1) * cfg.GROUP, cfg.NW)):
                for b in range(cfg.nbins(w)):
                    ck = group_chunks.get((t, w, ri, h, b))
                    lst = ck[c] if ck else []
                    for d in range(cmax.get((t, w, ri, h, b), 0)):
                        if d < len(lst):
                            ii, dd, nn = lst[d]
                            s0 = j * 128
                            sl = np.arange(s0, s0 + len(ii))
                            idxa[sl % 16, call["icol"] + sl // 16] = ii
                        j += 1
        # msel fill: walk windows
        for (t, w), wd in windows.items():
            dupc = {}
            for ci, (ri, h, jc, poff) in enumerate(wd["chunks"]):
                b = poff // BINW
                ck = group_chunks.get((t, w, ri, h, b))
                lst = ck[c] if ck else []
                d = dupc.get((ri, h, poff), 0)
                dupc[(ri, h, poff)] = d + 1
                if d < len(lst):
                    ii, dd, nn = lst[d]
                    m0 = wd["mcol"] + ci * BINW
                    msel[np.arange(len(ii)), m0 + dd] = nn
        idx_full = np.tile(idxa, (8, 1))
        per_core.append(dict(idx=idx_full,
                             msel=msel.astype(ml_dtypes.bfloat16)))
    return meta, per_core


def _build_program(cfg, meta):
    """Build the SPMD Bass program (same for all cores)."""
    from concourse import bacc

    NC, WIN, BINW = cfg.NC, cfg.WIN, cfg.BINW
    nc = bacc.Bacc("TRN2", target_bir_lowering=False, debug=False,
                   num_devices=NC, num_swdge_queues=4)
    gq = [0]  # round-robin SWDGE queue counter for gathers

    # I/O
    xt = {}
    for sd, nm in ((0, "d"), (1, "s")):
        for h in (0, 1):
            xt[(sd, h)] = nc.dram_tensor(
                f"x_{nm}_h{h}", [cfg.HALF, 128], BF16, kind="ExternalInput")
    idx_d = nc.dram_tensor("idx", [128, meta["ICOLS"]], I16, kind="ExternalInput")
    msel_d = nc.dram_tensor("msel", [128, meta["MCOLS"]], BF16, kind="ExternalInput")
    wts_d = nc.dram_tensor("wts", [2, 2, 2, 128, 128], BF16, kind="ExternalInput")
    linwt_d = nc.dram_tensor("linwt", [128, cfg.OUT], BF16, kind="ExternalInput")
    bias1_d = nc.dram_tensor("bias1", [2, 128, 2 * 128], F32, kind="ExternalInput")
    bias2_d = nc.dram_tensor("bias2", [2, 128, 1], F32, kind="ExternalInput")
    linb_d = nc.dram_tensor("linb", [128, 2 * cfg.OUT], F32, kind="ExternalInput")
    out_d = nc.dram_tensor("out", [2 * cfg.S, cfg.OUT], F32, kind="ExternalOutput")

    z_loc = [nc.dram_tensor(f"z_loc{t}", [cfg.S, 128], BF16) for t in (0, 1)]
    z_full = [nc.dram_tensor(f"z_full{t}", [cfg.N, 128], BF16,
                             addr_space="Shared") for t in (0, 1)]
    z_hi = [nc.dram_tensor(f"z_hi{t}", [cfg.HALF, 128], BF16) for t in (0, 1)]

    calls, windows = meta["calls"], meta["windows"]
    call_lookup = meta["call_lookup"]

    with tile.TileContext(nc) as tc:
        cpool = tc.alloc_tile_pool(name="const", bufs=1)
        gpool = tc.alloc_tile_pool(name="gather", bufs=2)
        ipool = tc.alloc_tile_pool(name="idx", bufs=2)
        mpool = tc.alloc_tile_pool(name="msel", bufs=2)
        apool = tc.alloc_tile_pool(name="aggs", bufs=2)
        zpool = tc.alloc_tile_pool(name="z", bufs=2)
        pagg = tc.alloc_tile_pool(name="pagg", bufs=2, space="PSUM")
        ptrf = tc.alloc_tile_pool(name="ptrf", bufs=2, space="PSUM")
        pfin = tc.alloc_tile_pool(name="pfin", bufs=2, space="PSUM")

        # constants to SBUF
        wsb = {}
        for l in (0, 1):
            for t in (0, 1):
                for ri in (0, 1):
                    w = cpool.tile([128, 128], BF16, tag=f"w{l}{t}{ri}", name=f"w_{l}{t}{ri}")
                    nc.sync.dma_start(w[:], wts_d[l, t, ri])
                    wsb[(l, t, ri)] = w
        linwt = cpool.tile([128, cfg.OUT], BF16, tag="linwt")
        nc.sync.dma_start(linwt[:], linwt_d[:])
        b1 = {}
        b2 = {}
        for t in (0, 1):
            b1[t] = cpool.tile([128, 256], F32, tag=f"b1{t}", name=f"b1_{t}")
            nc.sync.dma_start(b1[t][:], bias1_d[t])
            b2[t] = cpool.tile([128, 1], F32, tag=f"b2{t}", name=f"b2_{t}")
            nc.sync.dma_start(b2[t][:], bias2_d[t])
        linb = cpool.tile([128, 2 * cfg.OUT], F32, tag="linb")
        nc.sync.dma_start(linb[:], linb_d[:])

        def gather_tables(l, sd):
            if l == 0:
                return [xt[(sd, 0)].ap(), xt[(sd, 1)].ap()]
            return [z_full[sd][0:cfg.HALF, :], z_hi[sd].ap()]

        def do_layer(l):
            gtiles = {}
            for t in (0, 1):
                for w in range(cfg.NW):
                    ws = cfg.win_size(w)
                    g = w // cfg.GROUP
                    if w % cfg.GROUP == 0:
                        for ri in range(2):
                            for h in (0, 1):
                                call = calls[call_lookup[(t, g, ri, h)]]
                                C = call["nchunks"]
                                if C == 0:
                                    gtiles[(ri, h)] = None
                                    continue
                                it = ipool.tile([128, C * 8], I16, tag=f"i{ri}{h}", name=f"it{ri}{h}")
                                nc.sync.dma_start(
                                    it[:], idx_d[:, call["icol"]:call["icol"] + C * 8])
                                gt = gpool.tile([128, C, 128], BF16, tag=f"g{ri}{h}", name=f"gt{ri}{h}")
                                src = gather_tables(l, call["src_dis"])[h]
                                if not getattr(cfg, "no_gather", False):
                                    for j0 in range(0, C, 8):
                                        cs = min(8, C - j0)
                                        nc.gpsimd.dma_gather(
                                            gt[:, j0:j0 + cs, :], src,
                                            it[:, j0 * 8:(j0 + cs) * 8],
                                            cs * 128, cs * 128, 128,
                                            queue_num=gq[0] % 4)
                                        gq[0] += 1
                                else:
                                    nc.vector.memset(gt[:, 0, :], 0.0)
                                gtiles[(ri, h)] = gt
                    wd = windows[(t, w)]
                    nch = len(wd["chunks"])
                    if nch:
                        mt = mpool.tile([128, nch * BINW], BF16, tag="m", name="mt")
                        if not getattr(cfg, "no_msel", False):
                            nc.sync.dma_start(
                                mt[:], msel_d[:, wd["mcol"]:wd["mcol"] + nch * BINW])
                        else:
                            nc.vector.memset(mt[:, 0:BINW], 0.0)
                    aggP = [pagg.tile([128, 256], F32, tag=f"agg{r}", name=f"aggP{r}") for r in (0, 1)]
                    nc.vector.memset(aggP[0][:], 0.0)
                    nc.vector.memset(aggP[1][:], 0.0)
                    last_of_r = {}
                    for ci, (ri, h, j, poff) in enumerate(wd["chunks"]):
                        last_of_r[ri] = ci
                    if not getattr(cfg, "no_mm", False):
                        for ci, (ri, h, j, poff) in enumerate(wd["chunks"]):
                            gt = gtiles[(ri, h)]
                            nc.tensor.matmul(
                                aggP[ri][:, poff:poff + BINW],
                                gt[:, j, :],
                                mt[:, ci * BINW:(ci + 1) * BINW],
                                start=False, stop=(last_of_r[ri] == ci),
                                skip_group_check=True)
                    aggS = []
                    for r in (0, 1):
                        a = apool.tile([128, 256], BF16, tag=f"as{r}", name=f"aggS{r}")
                        nc.vector.tensor_copy(a[:, 0:ws], aggP[r][:, 0:ws])
                        aggS.append(a)
                    if l == 0:
                        outP = ptrf.tile([128, 256], F32, tag="tp")
                        nsub = (ws + 127) // 128
                        for j in range(nsub):
                            m = min(128, ws - j * 128)
                            nc.tensor.matmul(
                                outP[0:m, j * 128:j * 128 + 128],
                                aggS[0][:, j * 128:j * 128 + m],
                                wsb[(l, t, 0)][:], start=True, stop=False)
                            nc.tensor.matmul(
                                outP[0:m, j * 128:j * 128 + 128],
                                aggS[1][:, j * 128:j * 128 + m],
                                wsb[(l, t, 1)][:], start=False, stop=True)
                        tmp = zpool.tile([128, 256], F32, tag="tmp")
                        zsb = zpool.tile([128, 256], BF16, tag="zsb")
                        for j in range(nsub):
                            m = min(128, ws - j * 128)
                            sl = slice(j * 128, j * 128 + 128)
                            nc.vector.tensor_add(tmp[0:m, sl], outP[0:m, sl],
                                                 b1[t][0:m, sl])
                            nc.scalar.activation(
                                zsb[0:m, sl], tmp[0:m, sl],
                                mybir.ActivationFunctionType.Relu)
                            base = w * WIN + j * 128
                            nc.sync.dma_start(
                                z_loc[t][base:base + m, :], zsb[0:m, sl])
                    else:
                        z2P = ptrf.tile([128, 256], F32, tag="tp")
                        nc.tensor.matmul(z2P[:, 0:ws], wsb[(l, t, 0)][:],
                                         aggS[0][:, 0:ws], start=True, stop=False)
                        nc.tensor.matmul(z2P[:, 0:ws], wsb[(l, t, 1)][:],
                                         aggS[1][:, 0:ws], start=False, stop=True)
                        z2T = zpool.tile([128, 256], BF16, tag="z2t")
                        nc.scalar.activation(z2T[:, 0:ws], z2P[:, 0:ws],
                                             mybir.ActivationFunctionType.Relu,
                                             bias=b2[t][:])
                        fP = pfin.tile([128, 128], F32, tag="fp")
                        nsub = (ws + 127) // 128
                        for j in range(nsub):
                            m = min(128, ws - j * 128)
                            nc.tensor.matmul(
                                fP[0:m, j * 64:j * 64 + 64],
                                z2T[:, j * 128:j * 128 + m],
                                linwt[:], start=True, stop=True)
                        fo = zpool.tile([128, 128], F32, tag="fo")
                        for j in range(nsub):
                            m = min(128, ws - j * 128)
                            sl = slice(j * 64, j * 64 + 64)
                            nc.vector.tensor_add(fo[0:m, sl], fP[0:m, sl],
                                                 linb[0:m, sl])
                            base = t * cfg.S + w * WIN + j * 128
                            nc.sync.dma_start(out_d[base:base + m, :], fo[0:m, sl])

        stage = getattr(cfg, "stage", 2)
        do_layer(0)
        for t in (0, 1):
            if stage < 1:
                break
            nc.gpsimd.collective_compute(
                "AllGather", mybir.AluOpType.bypass,
                replica_groups=[list(range(NC))],
                ins=[z_loc[t].ap().opt()], outs=[z_full[t].ap().opt()])
        if stage >= 1:
            tc.strict_bb_all_engine_barrier()
            for t in (0, 1):
                nc.sync.dma_start(z_hi[t].ap(), z_full[t][cfg.HALF:2 * cfg.HALF, :])
        if stage >= 2:
            do_layer(1)

        for p in (pfin, ptrf, pagg, zpool, apool, mpool, ipool, gpool, cpool):
            p.release()

    nc.compile()
    return nc


def _make_inputs(cfg, per_core, x_drug, x_dis, Ws, bs, lin_w, lin_b):
    bf = ml_dtypes.bfloat16
    xb = {0: x_drug.astype(bf), 1: x_dis.astype(bf)}
    wts = np.zeros((2, 2, 2, 128, 128), np.float32)
    b1 = np.zeros((2, 128, 256), np.float32)
    b2 = np.zeros((2, 128, 1), np.float32)
    for l in (0, 1):
        for t in (0, 1):
            for ri in (0, 1):
                r = REL_OF_T[t][ri][0]
                wts[l, t, ri] = Ws[l, r]
            bsum = bs[l, REL_OF_T[t][0][0]] + bs[l, REL_OF_T[t][1][0]]
            if l == 0:
                b1[t] = np.tile(bsum[None, :], (128, 2))
            else:
                b2[t] = bsum[:, None].astype(np.float32)
    shared = {
        "wts": wts.astype(bf),
        "linwt": lin_w.T.astype(bf).copy(),
        "bias1": b1,
        "bias2": b2,
        "linb": np.tile(lin_b[None, :], (128, 2)).astype(np.float32),
    }
    for sd, nm in ((0, "d"), (1, "s")):
        for h in (0, 1):
            shared[f"x_{nm}_h{h}"] = np.ascontiguousarray(
                xb[sd][h * cfg.HALF:(h + 1) * cfg.HALF])
    in_maps = []
    for c in range(cfg.NC):
        m = dict(shared)
        m["idx"] = per_core[c]["idx"]
        m["msel"] = per_core[c]["msel"]
        in_maps.append(m)
    return in_maps


def run(cfg, x_drug, x_dis, eis, Ws, bs, lin_w, lin_b, trace=False):
    edge_arrays = {r: (eis[r][0].astype(np.int64), eis[r][1].astype(np.int64))
                   for r in range(4)}
    meta, per_core = _prep_graph(cfg, edge_arrays)
    nc = _build_program(cfg, meta)
    in_maps = _make_inputs(cfg, per_core, x_drug, x_dis, Ws, bs, lin_w, lin_b)
    res = run_bass_kernel_spmd(nc, in_maps, core_ids=list(range(cfg.NC)),
                               trace=trace)
    drug = np.zeros((cfg.N, cfg.OUT), np.float32)
    dis = np.zeros((cfg.N, cfg.OUT), np.float32)
    for c in range(cfg.NC):
        o = res.results[c]["out"]
        drug[c * cfg.S:(c + 1) * cfg.S] = o[:cfg.S]
        dis[c * cfg.S:(c + 1) * cfg.S] = o[cfg.S:]
    return (drug, dis), res


def kernel(x_drug, x_dis, ei_dd, ei_ss, ei_ds, ei_sd, Ws, bs, lin_w, lin_b):
    cfg = Cfg()
    eis = {0: np.asarray(ei_dd), 1: np.asarray(ei_ss),
           2: np.asarray(ei_ds), 3: np.asarray(ei_sd)}
    out, _ = run(cfg, np.asarray(x_drug), np.asarray(x_dis), eis,
                 np.asarray(Ws), np.asarray(bs),
                 np.asarray(lin_w), np.asarray(lin_b))
    return out



# revision 15
# speedup vs baseline: 2.2712x; 1.0912x over previous
"""HeteroGNN (2-layer hetero GCN) Trainium2 kernel, 8-core SPMD.

Strategy: destination-sharded. Each core owns 6250 drug + 6250 dis nodes.
Feature tables (bf16 rows) live in per-core HBM; edge gathers use
dma_gather (custom SWDGE row gather); scatter-add is done as one-hot
"Msel" matmuls accumulating in PSUM (edges chunked 128 at a time, each
chunk's destinations confined to a 32-wide bin so PSUM offsets are
program constants shared by all cores). Layer-1 output slices are
exchanged with two AllGather collectives, then layer 2 + final linear.
All graph preprocessing (degrees, norms, chunking, padding to the
max-over-cores schedule) happens on host in numpy.
"""

import numpy as np
import ml_dtypes

import sys

for _p in ("/opt/trn_rl_repo",):
    if _p not in sys.path:
        sys.path.insert(0, _p)

import concourse.bass as bass
import concourse.mybir as mybir
from concourse import tile
from concourse.bass_utils import run_bass_kernel_spmd

BF16 = mybir.dt.bfloat16
F32 = mybir.dt.float32
I16 = mybir.dt.int16


class Cfg:
    def __init__(self, n=50000, e=800000, ncores=8, win=256, binw=32, group=2):
        self.N = n              # nodes per type
        self.E = e              # edges per relation
        self.NC = ncores
        self.S = n // ncores    # dst nodes per core per type
        self.WIN = win          # dsts per PSUM window
        self.BINW = binw        # dsts per bin (fixed psum offset granularity)
        self.GROUP = group      # windows per gather call
        self.NW = (self.S + win - 1) // win   # windows per type
        self.NG = (self.NW + group - 1) // group
        self.HALF = n // 2      # rows per gather half-table (int16 idx limit)
        assert self.HALF <= 32768
        self.D = 128
        self.OUT = 64

    def win_size(self, w):
        return min(self.WIN, self.S - w * self.WIN)

    def nbins(self, w):
        ws = self.win_size(w)
        return (ws + self.BINW - 1) // self.BINW


# relations per dst type: (reference rel index, src_is_dis)
# drug dst: rel 0 (dd, src drug), rel 3 (sd, src dis)
# dis  dst: rel 1 (ss, src dis),  rel 2 (ds, src drug)
REL_OF_T = {0: [(0, 0), (3, 1)], 1: [(1, 1), (2, 0)]}
SELF_LOOP = {0: True, 1: True, 2: False, 3: False}


def _balance_perm(cfg, edge_arrays):
    """Per dst type: degree-balanced node->slot permutation (slot=core*S+d_loc).
    Deals nodes (sorted by total inbound degree) across bins card-style so
    every (core, window, bin) cell gets a near-equal edge load, shrinking the
    max-over-cores chunk padding."""
    perms = {}
    for t in (0, 1):
        deg = np.zeros(cfg.N, np.int64)
        for (r, src_dis) in REL_OF_T[t]:
            row, col = edge_arrays[r]
            deg += np.bincount(col, minlength=cfg.N)
            if SELF_LOOP[r]:
                deg += 1
        order = np.argsort(-deg, kind="stable")
        caps = []
        for c in range(cfg.NC):
            for w in range(cfg.NW):
                ws = cfg.win_size(w)
                for b in range(cfg.nbins(w)):
                    bw = min(cfg.BINW, ws - b * cfg.BINW)
                    caps.append((c * cfg.S + w * cfg.WIN + b * cfg.BINW, bw))
        slots = []
        for pos in range(cfg.BINW):
            for base, bw in caps:
                if pos < bw:
                    slots.append(base + pos)
        perm = np.empty(cfg.N, np.int64)
        perm[order] = np.asarray(slots, np.int64)
        perms[t] = perm
    return perms


def _prep_graph(cfg, edge_arrays):
    """edge_arrays: dict rel_idx -> (row, col) int64 full edge lists.
    Returns (meta, per_core) where meta is the SPMD-uniform schedule and
    per_core[c] = dict(idx=int16 [128, ICOLS], msel=f32 [128, MCOLS])."""
    N, S, WIN, BINW, NC = cfg.N, cfg.S, cfg.WIN, cfg.BINW, cfg.NC

    perms = _balance_perm(cfg, edge_arrays)

    # chunks[(t, w, r, h, b)][core] = list of (idx128 array, dloc array, norm array)
    group_chunks = {}
    for t in (0, 1):
        for ri, (r, src_dis) in enumerate(REL_OF_T[t]):
            row, col = edge_arrays[r]
            if SELF_LOOP[r]:
                sl = np.arange(N, dtype=np.int64)
                row = np.concatenate([row, sl])
                col = np.concatenate([col, sl])
            deg_s = np.bincount(row, minlength=N).astype(np.float64)
            deg_d = np.bincount(col, minlength=N).astype(np.float64)
            norm = (deg_s[row] ** -0.5 * deg_d[col] ** -0.5).astype(np.float32)
            row = perms[src_dis][row]   # source rows in slot order
            col = perms[t][col]         # dst in slot order
            core = col // S
            d_loc = col % S
            w = d_loc // WIN
            b = (d_loc % WIN) // BINW
            h = row // cfg.HALF
            idx16 = (row % cfg.HALF).astype(np.int16)
            # group key: (core, w, b, h)
            nb_max = (WIN + BINW - 1) // BINW
            key = ((core * cfg.NW + w) * nb_max + b) * 2 + h
            order = np.argsort(key, kind="stable")
            key_s = key[order]
            uk, starts = np.unique(key_s, return_index=True)
            starts = list(starts) + [len(key_s)]
            for gi, k in enumerate(uk):
                sl_ = order[starts[gi]:starts[gi + 1]]
                kk = int(k)
                hh = kk % 2
                kk //= 2
                bb = kk % nb_max
                kk //= nb_max
                ww = kk % cfg.NW
                cc = kk // cfg.NW
                gkey = (t, ww, ri, hh, bb)
                group_chunks.setdefault(gkey, {c: [] for c in range(NC)})
                lst = group_chunks[gkey][cc]
                for s0 in range(0, len(sl_), 128):
                    ee = sl_[s0:s0 + 128]
                    lst.append((idx16[ee], (d_loc[ee] % WIN) % BINW, norm[ee]))

    # C_max per slot key
    cmax = {}
    for gkey, bycore in group_chunks.items():
        cmax[gkey] = max(len(v) for v in bycore.values())

    # Build uniform schedule.
    # calls: per (t, g, ri, h): list of chunk slot keys in order (w asc, b asc, dup)
    calls = []           # (t, g, ri, h, src_dis, idx_col_off, nchunks)
    call_lookup = {}     # (t, g, ri, h) -> call index
    icol = 0
    for t in (0, 1):
        for g in range(cfg.NG):
            for ri in range(2):
                src_dis = REL_OF_T[t][ri][1]
                for h in (0, 1):
                    nch = 0
                    for w in range(g * cfg.GROUP, min((g + 1) * cfg.GROUP, cfg.NW)):
                        for b in range(cfg.nbins(w)):
                            nch += cmax.get((t, w, ri, h, b), 0)
                    call_lookup[(t, g, ri, h)] = len(calls)
                    calls.append(dict(t=t, g=g, ri=ri, h=h, src_dis=src_dis,
                                      icol=icol, nchunks=nch))
                    icol += nch * 8
    ICOLS = max(icol, 8)

    # windows: per (t, w): msel col offset + chunk list
    windows = {}
    mcol = 0
    for t in (0, 1):
        for w in range(cfg.NW):
            wch = []   # (ri, h, j_in_call, psum_off)
            # j_in_call accumulators per (ri, h) for this group
            for ri in range(2):
                for h in (0, 1):
                    j = 0
                    g = w // cfg.GROUP
                    for w2 in range(g * cfg.GROUP, w):
                        for b in range(cfg.nbins(w2)):
                            j += cmax.get((t, w2, ri, h, b), 0)
                    for b in range(cfg.nbins(w)):
                        for d in range(cmax.get((t, w, ri, h, b), 0)):
                            wch.append((ri, h, j, b * BINW))
                            j += 1
            windows[(t, w)] = dict(mcol=mcol, chunks=wch)
            mcol += len(wch) * BINW
    MCOLS = max(mcol, BINW)

    meta = dict(calls=calls, call_lookup=call_lookup, windows=windows,
                ICOLS=ICOLS, MCOLS=MCOLS, perms=perms)

    # Per-core data arrays
    per_core = []
    for c in range(NC):
        idxa = np.zeros((16, ICOLS), np.int16)
        msel = np.zeros((128, MCOLS), np.float32)
        for call in calls:
            t, g, ri, h = call["t"], call["g"], call["ri"], call["h"]
            j = 0
            for w in range(g * cfg.GROUP, min((g + 1) * cfg.GROUP, cfg.NW)):
                for b in range(cfg.nbins(w)):
                    ck = group_chunks.get((t, w, ri, h, b))
                    lst = ck[c] if ck else []
                    for d in range(cmax.get((t, w, ri, h, b), 0)):
                        if d < len(lst):
                            ii, dd, nn = lst[d]
                            s0 = j * 128
                            sl = np.arange(s0, s0 + len(ii))
                            idxa[sl % 16, call["icol"] + sl // 16] = ii
                        j += 1
        # msel fill: walk windows
        for (t, w), wd in windows.items():
            dupc = {}
            for ci, (ri, h, jc, poff) in enumerate(wd["chunks"]):
                b = poff // BINW
                ck = group_chunks.get((t, w, ri, h, b))
                lst = ck[c] if ck else []
                d = dupc.get((ri, h, poff), 0)
                dupc[(ri, h, poff)] = d + 1
                if d < len(lst):
                    ii, dd, nn = lst[d]
                    m0 = wd["mcol"] + ci * BINW
                    msel[np.arange(len(ii)), m0 + dd] = nn
        idx_full = np.tile(idxa, (8, 1))
        per_core.append(dict(idx=idx_full,
                             msel=msel.astype(ml_dtypes.bfloat16)))
    return meta, per_core


def _build_program(cfg, meta):
    """Build the SPMD Bass program (same for all cores)."""
    from concourse import bacc

    NC, WIN, BINW = cfg.NC, cfg.WIN, cfg.BINW
    nc = bacc.Bacc("TRN2", target_bir_lowering=False, debug=False,
                   num_devices=NC, num_swdge_queues=4)
    gq = [0]  # round-robin SWDGE queue counter for gathers

    # I/O
    xt = {}
    for sd, nm in ((0, "d"), (1, "s")):
        for h in (0, 1):
            xt[(sd, h)] = nc.dram_tensor(
                f"x_{nm}_h{h}", [cfg.HALF, 128], BF16, kind="ExternalInput")
    idx_d = nc.dram_tensor("idx", [128, meta["ICOLS"]], I16, kind="ExternalInput")
    msel_d = nc.dram_tensor("msel", [128, meta["MCOLS"]], BF16, kind="ExternalInput")
    wts_d = nc.dram_tensor("wts", [2, 2, 2, 128, 128], BF16, kind="ExternalInput")
    linwt_d = nc.dram_tensor("linwt", [128, cfg.OUT], BF16, kind="ExternalInput")
    bias1_d = nc.dram_tensor("bias1", [2, 128, 2 * 128], F32, kind="ExternalInput")
    bias2_d = nc.dram_tensor("bias2", [2, 128, 1], F32, kind="ExternalInput")
    linb_d = nc.dram_tensor("linb", [128, 2 * cfg.OUT], F32, kind="ExternalInput")
    out_d = nc.dram_tensor("out", [2 * cfg.S, cfg.OUT], F32, kind="ExternalOutput")

    z_loc = [nc.dram_tensor(f"z_loc{t}", [cfg.S, 128], BF16) for t in (0, 1)]
    z_full = [nc.dram_tensor(f"z_full{t}", [cfg.N, 128], BF16,
                             addr_space="Shared") for t in (0, 1)]

    calls, windows = meta["calls"], meta["windows"]
    call_lookup = meta["call_lookup"]

    with tile.TileContext(nc) as tc:
        cpool = tc.alloc_tile_pool(name="const", bufs=1)
        gpool = tc.alloc_tile_pool(name="gather", bufs=2)
        ipool = tc.alloc_tile_pool(name="idx", bufs=2)
        mpool = tc.alloc_tile_pool(name="msel", bufs=2)
        apool = tc.alloc_tile_pool(name="aggs", bufs=2)
        zpool = tc.alloc_tile_pool(name="z", bufs=2)
        pagg = tc.alloc_tile_pool(name="pagg", bufs=2, space="PSUM")
        ptrf = tc.alloc_tile_pool(name="ptrf", bufs=2, space="PSUM")
        pfin = tc.alloc_tile_pool(name="pfin", bufs=2, space="PSUM")

        # constants to SBUF
        wsb = {}
        for l in (0, 1):
            for t in (0, 1):
                for ri in (0, 1):
                    w = cpool.tile([128, 128], BF16, tag=f"w{l}{t}{ri}", name=f"w_{l}{t}{ri}")
                    nc.sync.dma_start(w[:], wts_d[l, t, ri])
                    wsb[(l, t, ri)] = w
        linwt = cpool.tile([128, cfg.OUT], BF16, tag="linwt")
        nc.sync.dma_start(linwt[:], linwt_d[:])
        b1 = {}
        b2 = {}
        for t in (0, 1):
            b1[t] = cpool.tile([128, 256], F32, tag=f"b1{t}", name=f"b1_{t}")
            nc.sync.dma_start(b1[t][:], bias1_d[t])
            b2[t] = cpool.tile([128, 1], F32, tag=f"b2{t}", name=f"b2_{t}")
            nc.sync.dma_start(b2[t][:], bias2_d[t])
        linb = cpool.tile([128, 2 * cfg.OUT], F32, tag="linb")
        nc.sync.dma_start(linb[:], linb_d[:])

        def gather_tables(l, sd):
            if l == 0:
                return [xt[(sd, 0)].ap(), xt[(sd, 1)].ap()]
            return [z_full[sd][0:cfg.HALF, :], z_full[sd][cfg.HALF:2 * cfg.HALF, :]]

        def do_layer(l, t, mid_hook=None):
            gtiles = {}
            if True:
                for w in range(cfg.NW):
                    ws = cfg.win_size(w)
                    g = w // cfg.GROUP
                    if w % cfg.GROUP == 0:
                        if mid_hook is not None:
                            mid_hook(g)
                        for ri in range(2):
                            for h in (0, 1):
                                call = calls[call_lookup[(t, g, ri, h)]]
                                C = call["nchunks"]
                                if C == 0:
                                    gtiles[(ri, h)] = None
                                    continue
                                it = ipool.tile([128, C * 8], I16, tag=f"i{ri}{h}", name=f"it{ri}{h}")
                                nc.sync.dma_start(
                                    it[:], idx_d[:, call["icol"]:call["icol"] + C * 8])
                                gt = gpool.tile([128, C, 128], BF16, tag=f"g{ri}{h}", name=f"gt{ri}{h}")
                                src = gather_tables(l, call["src_dis"])[h]
                                if not getattr(cfg, "no_gather", False):
                                    for j0 in range(0, C, 8):
                                        cs = min(8, C - j0)
                                        nc.gpsimd.dma_gather(
                                            gt[:, j0:j0 + cs, :], src,
                                            it[:, j0 * 8:(j0 + cs) * 8],
                                            cs * 128, cs * 128, 128,
                                            queue_num=gq[0] % 4)
                                        gq[0] += 1
                                else:
                                    nc.vector.memset(gt[:, 0, :], 0.0)
                                gtiles[(ri, h)] = gt
                    wd = windows[(t, w)]
                    nch = len(wd["chunks"])
                    if nch:
                        mt = mpool.tile([128, nch * BINW], BF16, tag="m", name="mt")
                        if not getattr(cfg, "no_msel", False):
                            nc.sync.dma_start(
                                mt[:], msel_d[:, wd["mcol"]:wd["mcol"] + nch * BINW])
                        else:
                            nc.vector.memset(mt[:, 0:BINW], 0.0)
                    aggP = [pagg.tile([128, 256], F32, tag=f"agg{r}", name=f"aggP{r}") for r in (0, 1)]
                    nc.vector.memset(aggP[0][:], 0.0)
                    nc.vector.memset(aggP[1][:], 0.0)
                    last_of_r = {}
                    for ci, (ri, h, j, poff) in enumerate(wd["chunks"]):
                        last_of_r[ri] = ci
                    if not getattr(cfg, "no_mm", False):
                        for ci, (ri, h, j, poff) in enumerate(wd["chunks"]):
                            gt = gtiles[(ri, h)]
                            nc.tensor.matmul(
                                aggP[ri][:, poff:poff + BINW],
                                gt[:, j, :],
                                mt[:, ci * BINW:(ci + 1) * BINW],
                                start=False, stop=(last_of_r[ri] == ci),
                                skip_group_check=True)
                    aggS = []
                    for r in (0, 1):
                        a = apool.tile([128, 256], BF16, tag=f"as{r}", name=f"aggS{r}")
                        nc.vector.tensor_copy(a[:, 0:ws], aggP[r][:, 0:ws])
                        aggS.append(a)
                    if l == 0:
                        outP = ptrf.tile([128, 256], F32, tag="tp")
                        nsub = (ws + 127) // 128
                        for j in range(nsub):
                            m = min(128, ws - j * 128)
                            nc.tensor.matmul(
                                outP[0:m, j * 128:j * 128 + 128],
                                aggS[0][:, j * 128:j * 128 + m],
                                wsb[(l, t, 0)][:], start=True, stop=False)
                            nc.tensor.matmul(
                                outP[0:m, j * 128:j * 128 + 128],
                                aggS[1][:, j * 128:j * 128 + m],
                                wsb[(l, t, 1)][:], start=False, stop=True)
                        tmp = zpool.tile([128, 256], F32, tag="tmp")
                        zsb = zpool.tile([128, 256], BF16, tag="zsb")
                        for j in range(nsub):
                            m = min(128, ws - j * 128)
                            sl = slice(j * 128, j * 128 + 128)
                            nc.vector.tensor_add(tmp[0:m, sl], outP[0:m, sl],
                                                 b1[t][0:m, sl])
                            nc.scalar.activation(
                                zsb[0:m, sl], tmp[0:m, sl],
                                mybir.ActivationFunctionType.Relu)
                            base = w * WIN + j * 128
                            nc.sync.dma_start(
                                z_loc[t][base:base + m, :], zsb[0:m, sl])
                    else:
                        z2P = ptrf.tile([128, 256], F32, tag="tp")
                        nc.tensor.matmul(z2P[:, 0:ws], wsb[(l, t, 0)][:],
                                         aggS[0][:, 0:ws], start=True, stop=False)
                        nc.tensor.matmul(z2P[:, 0:ws], wsb[(l, t, 1)][:],
                                         aggS[1][:, 0:ws], start=False, stop=True)
                        z2T = zpool.tile([128, 256], BF16, tag="z2t")
                        nc.scalar.activation(z2T[:, 0:ws], z2P[:, 0:ws],
                                             mybir.ActivationFunctionType.Relu,
                                             bias=b2[t][:])
                        fP = pfin.tile([128, 128], F32, tag="fp")
                        nsub = (ws + 127) // 128
                        for j in range(nsub):
                            m = min(128, ws - j * 128)
                            nc.tensor.matmul(
                                fP[0:m, j * 64:j * 64 + 64],
                                z2T[:, j * 128:j * 128 + m],
                                linwt[:], start=True, stop=True)
                        fo = zpool.tile([128, 128], F32, tag="fo")
                        for j in range(nsub):
                            m = min(128, ws - j * 128)
                            sl = slice(j * 64, j * 64 + 64)
                            nc.vector.tensor_add(fo[0:m, sl], fP[0:m, sl],
                                                 linb[0:m, sl])
                            base = t * cfg.S + w * WIN + j * 128
                            nc.sync.dma_start(out_d[base:base + m, :], fo[0:m, sl])

        def do_ag(t):
            nc.gpsimd.collective_compute(
                "AllGather", mybir.AluOpType.bypass,
                replica_groups=[list(range(NC))],
                ins=[z_loc[t].ap().opt()], outs=[z_full[t].ap().opt()])

        stage = getattr(cfg, "stage", 2)
        do_layer(0, 0)
        hook = None
        if stage >= 1:
            hook = lambda g: do_ag(0) if g == 4 else None
        do_layer(0, 1, mid_hook=hook)
        if stage >= 1:
            do_ag(1)
        if stage >= 2:
            do_layer(1, 0)
            do_layer(1, 1)

        for p in (pfin, ptrf, pagg, zpool, apool, mpool, ipool, gpool, cpool):
            p.release()

    nc.compile()
    return nc


def _make_inputs(cfg, meta, per_core, x_drug, x_dis, Ws, bs, lin_w, lin_b):
    bf = ml_dtypes.bfloat16
    perms = meta["perms"]
    xs = {0: np.asarray(x_drug), 1: np.asarray(x_dis)}
    xb = {}
    for sd in (0, 1):
        xp = np.empty_like(xs[sd])
        xp[perms[sd]] = xs[sd]      # row slot s holds node with perm[node]==s
        xb[sd] = xp.astype(bf)
    wts = np.zeros((2, 2, 2, 128, 128), np.float32)
    b1 = np.zeros((2, 128, 256), np.float32)
    b2 = np.zeros((2, 128, 1), np.float32)
    for l in (0, 1):
        for t in (0, 1):
            for ri in (0, 1):
                r = REL_OF_T[t][ri][0]
                wts[l, t, ri] = Ws[l, r]
            bsum = bs[l, REL_OF_T[t][0][0]] + bs[l, REL_OF_T[t][1][0]]
            if l == 0:
                b1[t] = np.tile(bsum[None, :], (128, 2))
            else:
                b2[t] = bsum[:, None].astype(np.float32)
    shared = {
        "wts": wts.astype(bf),
        "linwt": lin_w.T.astype(bf).copy(),
        "bias1": b1,
        "bias2": b2,
        "linb": np.tile(lin_b[None, :], (128, 2)).astype(np.float32),
    }
    for sd, nm in ((0, "d"), (1, "s")):
        for h in (0, 1):
            shared[f"x_{nm}_h{h}"] = np.ascontiguousarray(
                xb[sd][h * cfg.HALF:(h + 1) * cfg.HALF])
    in_maps = []
    for c in range(cfg.NC):
        m = dict(shared)
        m["idx"] = per_core[c]["idx"]
        m["msel"] = per_core[c]["msel"]
        in_maps.append(m)
    return in_maps


def run(cfg, x_drug, x_dis, eis, Ws, bs, lin_w, lin_b, trace=False):
    edge_arrays = {r: (eis[r][0].astype(np.int64), eis[r][1].astype(np.int64))
                   for r in range(4)}
    meta, per_core = _prep_graph(cfg, edge_arrays)
    nc = _build_program(cfg, meta)
    in_maps = _make_inputs(cfg, meta, per_core, x_drug, x_dis, Ws, bs,
                           lin_w, lin_b)
    res = run_bass_kernel_spmd(nc, in_maps, core_ids=list(range(cfg.NC)),
                               trace=trace)
    drug, dis = assemble(cfg, meta, [res.results[c]["out"]
                                     for c in range(cfg.NC)])
    return (drug, dis), res


def assemble(cfg, meta, outs):
    """outs[c] = per-core [2*S, OUT] slot-ordered output -> full node order."""
    slotted = {0: np.zeros((cfg.N, cfg.OUT), np.float32),
               1: np.zeros((cfg.N, cfg.OUT), np.float32)}
    for c in range(cfg.NC):
        o = outs[c]
        slotted[0][c * cfg.S:(c + 1) * cfg.S] = o[:cfg.S]
        slotted[1][c * cfg.S:(c + 1) * cfg.S] = o[cfg.S:]
    return (slotted[0][meta["perms"][0]], slotted[1][meta["perms"][1]])


def kernel(x_drug, x_dis, ei_dd, ei_ss, ei_ds, ei_sd, Ws, bs, lin_w, lin_b):
    cfg = Cfg()
    eis = {0: np.asarray(ei_dd), 1: np.asarray(ei_ss),
           2: np.asarray(ei_ds), 3: np.asarray(ei_sd)}
    out, _ = run(cfg, np.asarray(x_drug), np.asarray(x_dis), eis,
                 np.asarray(Ws), np.asarray(bs),
                 np.asarray(lin_w), np.asarray(lin_b))
    return out



# revision 17
# speedup vs baseline: 5.2812x; 2.3253x over previous
"""HeteroGNN (2-layer hetero GCN) Trainium2 kernel, 8-core SPMD.

Strategy: destination-sharded. Each core owns 6250 drug + 6250 dis nodes.
Feature tables (bf16 rows) live in per-core HBM; edge gathers use
dma_gather (custom SWDGE row gather); scatter-add is done as one-hot
"Msel" matmuls accumulating in PSUM (edges chunked 128 at a time, each
chunk's destinations confined to a 32-wide bin so PSUM offsets are
program constants shared by all cores). Layer-1 output slices are
exchanged with two AllGather collectives, then layer 2 + final linear.
All graph preprocessing (degrees, norms, chunking, padding to the
max-over-cores schedule) happens on host in numpy.
"""

import numpy as np
import ml_dtypes

import sys

for _p in ("/opt/trn_rl_repo",):
    if _p not in sys.path:
        sys.path.insert(0, _p)

import concourse.bass as bass
import concourse.mybir as mybir
from concourse import tile
from concourse.bass_utils import run_bass_kernel_spmd

BF16 = mybir.dt.bfloat16
F32 = mybir.dt.float32
I16 = mybir.dt.int16


class Cfg:
    def __init__(self, n=50000, e=800000, ncores=8, win=256, binw=32, group=2):
        self.N = n              # nodes per type
        self.E = e              # edges per relation
        self.NC = ncores
        self.S = n // ncores    # dst nodes per core per type
        self.WIN = win          # dsts per PSUM window
        self.BINW = binw        # dsts per bin (fixed psum offset granularity)
        self.GROUP = group      # windows per gather call
        self.NW = (self.S + win - 1) // win   # windows per type
        self.NG = (self.NW + group - 1) // group
        self.HALF = n // 2      # rows per gather half-table (int16 idx limit)
        assert self.HALF <= 32768
        self.D = 128
        self.OUT = 64

    def win_size(self, w):
        return min(self.WIN, self.S - w * self.WIN)

    def nbins(self, w):
        ws = self.win_size(w)
        return (ws + self.BINW - 1) // self.BINW


# relations per dst type: (reference rel index, src_is_dis)
# drug dst: rel 0 (dd, src drug), rel 3 (sd, src dis)
# dis  dst: rel 1 (ss, src dis),  rel 2 (ds, src drug)
REL_OF_T = {0: [(0, 0), (3, 1)], 1: [(1, 1), (2, 0)]}
SELF_LOOP = {0: True, 1: True, 2: False, 3: False}


def _balance_perm(cfg, edge_arrays):
    """Per dst type: degree-balanced node->slot permutation (slot=core*S+d_loc).
    Deals nodes (sorted by total inbound degree) across bins card-style so
    every (core, window, bin) cell gets a near-equal edge load, shrinking the
    max-over-cores chunk padding."""
    perms = {}
    for t in (0, 1):
        deg = np.zeros(cfg.N, np.int64)
        for (r, src_dis) in REL_OF_T[t]:
            row, col = edge_arrays[r]
            deg += np.bincount(col, minlength=cfg.N)
            if SELF_LOOP[r]:
                deg += 1
        order = np.argsort(-deg, kind="stable")
        caps = []
        for c in range(cfg.NC):
            for w in range(cfg.NW):
                ws = cfg.win_size(w)
                for b in range(cfg.nbins(w)):
                    bw = min(cfg.BINW, ws - b * cfg.BINW)
                    caps.append((c * cfg.S + w * cfg.WIN + b * cfg.BINW, bw))
        slots = []
        for pos in range(cfg.BINW):
            for base, bw in caps:
                if pos < bw:
                    slots.append(base + pos)
        perm = np.empty(cfg.N, np.int64)
        perm[order] = np.asarray(slots, np.int64)
        perms[t] = perm
    return perms


def _prep_graph(cfg, edge_arrays):
    """edge_arrays: dict rel_idx -> (row, col) int64 full edge lists.
    Returns (meta, per_core) where meta is the SPMD-uniform schedule and
    per_core[c] = dict(idx=int16 [128, ICOLS], msel=bf16 [128, MCOLS]).

    Dense slot packing: per (t, group, ri, h) gather call, each core's edges
    are sorted by (window, bin) and packed 128 per slot with no per-bin
    padding. A slot spanning multiple bins gets one matmul piece per
    (bin, slot) overlap; msel blocks carry the per-core norms (zeros where a
    core has no edge for that piece)."""
    N, S, WIN, BINW, NC = cfg.N, cfg.S, cfg.WIN, cfg.BINW, cfg.NC
    NB_MAX = (WIN + BINW - 1) // BINW

    perms = _balance_perm(cfg, edge_arrays)

    # edata[(t, g, ri, h)][c] = dict(idx16, w, b, dpos, norm) sorted by (w, b)
    edata = {}
    for t in (0, 1):
        for ri, (r, src_dis) in enumerate(REL_OF_T[t]):
            row, col = edge_arrays[r]
            if SELF_LOOP[r]:
                sl = np.arange(N, dtype=np.int64)
                row = np.concatenate([row, sl])
                col = np.concatenate([col, sl])
            deg_s = np.bincount(row, minlength=N).astype(np.float64)
            deg_d = np.bincount(col, minlength=N).astype(np.float64)
            norm = (deg_s[row] ** -0.5 * deg_d[col] ** -0.5).astype(np.float32)
            row = perms[src_dis][row]   # source rows in slot order
            col = perms[t][col]         # dst in slot order
            core = col // S
            d_loc = col % S
            w = d_loc // WIN
            b = (d_loc % WIN) // BINW
            dpos = (d_loc % WIN) % BINW
            g = w // cfg.GROUP
            h = row // cfg.HALF
            idx16 = (row % cfg.HALF).astype(np.int16)
            order = np.lexsort((b, w, h, g, core))
            cs, gs, hs = core[order], g[order], h[order]
            seg_key = (cs * cfg.NG + gs) * 2 + hs
            uk, starts = np.unique(seg_key, return_index=True)
            starts = list(starts) + [len(seg_key)]
            for si, k in enumerate(uk):
                sl_ = order[starts[si]:starts[si + 1]]
                kk = int(k)
                hh = kk % 2
                kk //= 2
                gg = kk % cfg.NG
                cc = kk // cfg.NG
                edata.setdefault((t, gg, ri, hh), {})[cc] = dict(
                    idx16=idx16[sl_], w=w[sl_], b=b[sl_],
                    dpos=dpos[sl_], norm=norm[sl_])

    # calls: per (t, g, ri, h) one gather stream, nchunks = max-core slots
    calls = []
    call_lookup = {}
    icol = 0
    for t in (0, 1):
        for g in range(cfg.NG):
            for ri in range(2):
                src_dis = REL_OF_T[t][ri][1]
                for h in (0, 1):
                    segs = edata.get((t, g, ri, h), {})
                    C = 0
                    for c, seg in segs.items():
                        C = max(C, (len(seg["idx16"]) + 127) // 128)
                    call_lookup[(t, g, ri, h)] = len(calls)
                    calls.append(dict(t=t, g=g, ri=ri, h=h, src_dis=src_dis,
                                      icol=icol, nchunks=C))
                    icol += C * 8
    ICOLS = max(icol, 8)

    # per-seg, per-window slot ranges of each bin: cellrng[(t,g,ri,h,c)] =
    # dict[(w, b)] = (slot_lo, slot_hi, estart, eend)
    cellrng = {}
    for key, segs in edata.items():
        for c, seg in segs.items():
            wb = seg["w"].astype(np.int64) * NB_MAX + seg["b"]
            ub, st = np.unique(wb, return_index=True)
            st = list(st) + [len(wb)]
            d = {}
            for i, k in enumerate(ub):
                e0, e1 = st[i], st[i + 1]
                d[(int(k) // NB_MAX, int(k) % NB_MAX)] = (
                    e0 // 128, (e1 - 1) // 128, e0, e1)
            cellrng[key + (c,)] = d

    # windows: pieces = (ri, h, slot_j, poff); one piece per (bin, slot) in
    # the union-over-cores slot range of that bin
    windows = {}
    mcol = 0
    for t in (0, 1):
        for w in range(cfg.NW):
            g = w // cfg.GROUP
            pieces = []
            for ri in range(2):
                for h in (0, 1):
                    for b in range(cfg.nbins(w)):
                        lo, hi = None, None
                        for c in range(NC):
                            d = cellrng.get((t, g, ri, h, c))
                            if not d or (w, b) not in d:
                                continue
                            s0, s1, _, _ = d[(w, b)]
                            lo = s0 if lo is None else min(lo, s0)
                            hi = s1 if hi is None else max(hi, s1)
                        if lo is None:
                            continue
                        for j in range(lo, hi + 1):
                            pieces.append((ri, h, j, b * BINW))
            windows[(t, w)] = dict(mcol=mcol, chunks=pieces)
            mcol += len(pieces) * BINW
    MCOLS = max(mcol, BINW)

    meta = dict(calls=calls, call_lookup=call_lookup, windows=windows,
                ICOLS=ICOLS, MCOLS=MCOLS, perms=perms)

    # piece index lookup: (t, w) -> {(ri, h, j, b): ci}
    piece_ci = {}
    for (t, w), wd in windows.items():
        piece_ci[(t, w)] = {
            (ri, h, j, poff // BINW): ci
            for ci, (ri, h, j, poff) in enumerate(wd["chunks"])}

    # Per-core data arrays
    per_core = []
    for c in range(NC):
        idxa = np.zeros((16, ICOLS), np.int16)
        msel = np.zeros((128, MCOLS), np.float32)
        for call in calls:
            t, g, ri, h = call["t"], call["g"], call["ri"], call["h"]
            seg = edata.get((t, g, ri, h), {}).get(c)
            if seg is None:
                continue
            n = len(seg["idx16"])
            p = np.arange(n)
            idxa[p % 16, call["icol"] + p // 16] = seg["idx16"]
            # msel fill: per (w, b) cell, edges are contiguous [e0, e1)
            d = cellrng[(t, g, ri, h, c)]
            for (w, b), (s0, s1, e0, e1) in d.items():
                wd = windows[(t, w)]
                pc = piece_ci[(t, w)]
                ee = np.arange(e0, e1)
                slot = ee // 128
                par = ee % 128
                ci = pc[(ri, h, s0, b)] + (slot - s0)
                msel[par, wd["mcol"] + ci * BINW + seg["dpos"][e0:e1]] = \
                    seg["norm"][e0:e1]
        idx_full = np.tile(idxa, (8, 1))
        per_core.append(dict(idx=idx_full,
                             msel=msel.astype(ml_dtypes.bfloat16)))
    return meta, per_core


def _build_program(cfg, meta):
    """Build the SPMD Bass program (same for all cores)."""
    from concourse import bacc

    NC, WIN, BINW = cfg.NC, cfg.WIN, cfg.BINW
    nc = bacc.Bacc("TRN2", target_bir_lowering=False, debug=False,
                   num_devices=NC, num_swdge_queues=4)
    gq = [0]  # round-robin SWDGE queue counter for gathers

    # I/O
    xt = {}
    for sd, nm in ((0, "d"), (1, "s")):
        for h in (0, 1):
            xt[(sd, h)] = nc.dram_tensor(
                f"x_{nm}_h{h}", [cfg.HALF, 128], BF16, kind="ExternalInput")
    idx_d = nc.dram_tensor("idx", [128, meta["ICOLS"]], I16, kind="ExternalInput")
    msel_d = nc.dram_tensor("msel", [128, meta["MCOLS"]], BF16, kind="ExternalInput")
    wts_d = nc.dram_tensor("wts", [2, 2, 2, 128, 128], BF16, kind="ExternalInput")
    linwt_d = nc.dram_tensor("linwt", [128, cfg.OUT], BF16, kind="ExternalInput")
    bias1_d = nc.dram_tensor("bias1", [2, 128, 2 * 128], F32, kind="ExternalInput")
    bias2_d = nc.dram_tensor("bias2", [2, 128, 1], F32, kind="ExternalInput")
    linb_d = nc.dram_tensor("linb", [128, 2 * cfg.OUT], F32, kind="ExternalInput")
    out_d = nc.dram_tensor("out", [2 * cfg.S, cfg.OUT], F32, kind="ExternalOutput")

    z_loc = [nc.dram_tensor(f"z_loc{t}", [cfg.S, 128], BF16) for t in (0, 1)]
    z_full = [nc.dram_tensor(f"z_full{t}", [cfg.N, 128], BF16,
                             addr_space="Shared") for t in (0, 1)]

    calls, windows = meta["calls"], meta["windows"]
    call_lookup = meta["call_lookup"]

    with tile.TileContext(nc) as tc:
        cpool = tc.alloc_tile_pool(name="const", bufs=1)
        gpool = tc.alloc_tile_pool(name="gather", bufs=2)
        ipool = tc.alloc_tile_pool(name="idx", bufs=2)
        mpool = tc.alloc_tile_pool(name="msel", bufs=2)
        apool = tc.alloc_tile_pool(name="aggs", bufs=2)
        zpool = tc.alloc_tile_pool(name="z", bufs=2)
        pagg = tc.alloc_tile_pool(name="pagg", bufs=2, space="PSUM")
        ptrf = tc.alloc_tile_pool(name="ptrf", bufs=2, space="PSUM")
        pfin = tc.alloc_tile_pool(name="pfin", bufs=2, space="PSUM")

        # constants to SBUF
        wsb = {}
        for l in (0, 1):
            for t in (0, 1):
                for ri in (0, 1):
                    w = cpool.tile([128, 128], BF16, tag=f"w{l}{t}{ri}", name=f"w_{l}{t}{ri}")
                    nc.sync.dma_start(w[:], wts_d[l, t, ri])
                    wsb[(l, t, ri)] = w
        linwt = cpool.tile([128, cfg.OUT], BF16, tag="linwt")
        nc.sync.dma_start(linwt[:], linwt_d[:])
        b1 = {}
        b2 = {}
        for t in (0, 1):
            b1[t] = cpool.tile([128, 256], F32, tag=f"b1{t}", name=f"b1_{t}")
            nc.sync.dma_start(b1[t][:], bias1_d[t])
            b2[t] = cpool.tile([128, 1], F32, tag=f"b2{t}", name=f"b2_{t}")
            nc.sync.dma_start(b2[t][:], bias2_d[t])
        linb = cpool.tile([128, 2 * cfg.OUT], F32, tag="linb")
        nc.sync.dma_start(linb[:], linb_d[:])

        def gather_tables(l, sd):
            if l == 0:
                return [xt[(sd, 0)].ap(), xt[(sd, 1)].ap()]
            return [z_full[sd][0:cfg.HALF, :], z_full[sd][cfg.HALF:2 * cfg.HALF, :]]

        def do_layer(l, t, mid_hook=None):
            gtiles = {}
            if True:
                for w in range(cfg.NW):
                    ws = cfg.win_size(w)
                    g = w // cfg.GROUP
                    if w % cfg.GROUP == 0:
                        if mid_hook is not None:
                            mid_hook(g)
                        for ri in range(2):
                            for h in (0, 1):
                                call = calls[call_lookup[(t, g, ri, h)]]
                                C = call["nchunks"]
                                if C == 0:
                                    gtiles[(ri, h)] = None
                                    continue
                                it = ipool.tile([128, C * 8], I16, tag=f"i{ri}{h}", name=f"it{ri}{h}")
                                nc.sync.dma_start(
                                    it[:], idx_d[:, call["icol"]:call["icol"] + C * 8])
                                gt = gpool.tile([128, C, 128], BF16, tag=f"g{ri}{h}", name=f"gt{ri}{h}")
                                src = gather_tables(l, call["src_dis"])[h]
                                if not getattr(cfg, "no_gather", False):
                                    for j0 in range(0, C, 8):
                                        cs = min(8, C - j0)
                                        nc.gpsimd.dma_gather(
                                            gt[:, j0:j0 + cs, :], src,
                                            it[:, j0 * 8:(j0 + cs) * 8],
                                            cs * 128, cs * 128, 128,
                                            queue_num=gq[0] % 4)
                                        gq[0] += 1
                                else:
                                    nc.vector.memset(gt[:, 0, :], 0.0)
                                gtiles[(ri, h)] = gt
                    wd = windows[(t, w)]
                    nch = len(wd["chunks"])
                    if nch:
                        mt = mpool.tile([128, nch * BINW], BF16, tag="m", name="mt")
                        if not getattr(cfg, "no_msel", False):
                            nc.sync.dma_start(
                                mt[:], msel_d[:, wd["mcol"]:wd["mcol"] + nch * BINW])
                        else:
                            nc.vector.memset(mt[:, 0:BINW], 0.0)
                    aggP = [pagg.tile([128, 256], F32, tag=f"agg{r}", name=f"aggP{r}") for r in (0, 1)]
                    nc.vector.memset(aggP[0][:], 0.0)
                    nc.vector.memset(aggP[1][:], 0.0)
                    last_of_r = {}
                    for ci, (ri, h, j, poff) in enumerate(wd["chunks"]):
                        last_of_r[ri] = ci
                    if not getattr(cfg, "no_mm", False):
                        for ci, (ri, h, j, poff) in enumerate(wd["chunks"]):
                            gt = gtiles[(ri, h)]
                            nc.tensor.matmul(
                                aggP[ri][:, poff:poff + BINW],
                                gt[:, j, :],
                                mt[:, ci * BINW:(ci + 1) * BINW],
                                start=False, stop=(last_of_r[ri] == ci),
                                skip_group_check=True)
                    aggS = []
                    for r in (0, 1):
                        a = apool.tile([128, 256], BF16, tag=f"as{r}", name=f"aggS{r}")
                        nc.vector.tensor_copy(a[:, 0:ws], aggP[r][:, 0:ws])
                        aggS.append(a)
                    if l == 0:
                        outP = ptrf.tile([128, 256], F32, tag="tp")
                        nsub = (ws + 127) // 128
                        for j in range(nsub):
                            m = min(128, ws - j * 128)
                            nc.tensor.matmul(
                                outP[0:m, j * 128:j * 128 + 128],
                                aggS[0][:, j * 128:j * 128 + m],
                                wsb[(l, t, 0)][:], start=True, stop=False)
                            nc.tensor.matmul(
                                outP[0:m, j * 128:j * 128 + 128],
                                aggS[1][:, j * 128:j * 128 + m],
                                wsb[(l, t, 1)][:], start=False, stop=True)
                        tmp = zpool.tile([128, 256], F32, tag="tmp")
                        zsb = zpool.tile([128, 256], BF16, tag="zsb")
                        for j in range(nsub):
                            m = min(128, ws - j * 128)
                            sl = slice(j * 128, j * 128 + 128)
                            nc.vector.tensor_add(tmp[0:m, sl], outP[0:m, sl],
                                                 b1[t][0:m, sl])
                            nc.scalar.activation(
                                zsb[0:m, sl], tmp[0:m, sl],
                                mybir.ActivationFunctionType.Relu)
                            base = w * WIN + j * 128
                            nc.sync.dma_start(
                                z_loc[t][base:base + m, :], zsb[0:m, sl])
                    else:
                        z2P = ptrf.tile([128, 256], F32, tag="tp")
                        nc.tensor.matmul(z2P[:, 0:ws], wsb[(l, t, 0)][:],
                                         aggS[0][:, 0:ws], start=True, stop=False)
                        nc.tensor.matmul(z2P[:, 0:ws], wsb[(l, t, 1)][:],
                                         aggS[1][:, 0:ws], start=False, stop=True)
                        z2T = zpool.tile([128, 256], BF16, tag="z2t")
                        nc.scalar.activation(z2T[:, 0:ws], z2P[:, 0:ws],
                                             mybir.ActivationFunctionType.Relu,
                                             bias=b2[t][:])
                        fP = pfin.tile([128, 128], F32, tag="fp")
                        nsub = (ws + 127) // 128
                        for j in range(nsub):
                            m = min(128, ws - j * 128)
                            nc.tensor.matmul(
                                fP[0:m, j * 64:j * 64 + 64],
                                z2T[:, j * 128:j * 128 + m],
                                linwt[:], start=True, stop=True)
                        fo = zpool.tile([128, 128], F32, tag="fo")
                        for j in range(nsub):
                            m = min(128, ws - j * 128)
                            sl = slice(j * 64, j * 64 + 64)
                            nc.vector.tensor_add(fo[0:m, sl], fP[0:m, sl],
                                                 linb[0:m, sl])
                            base = t * cfg.S + w * WIN + j * 128
                            nc.sync.dma_start(out_d[base:base + m, :], fo[0:m, sl])

        def do_ag(t):
            nc.gpsimd.collective_compute(
                "AllGather", mybir.AluOpType.bypass,
                replica_groups=[list(range(NC))],
                ins=[z_loc[t].ap().opt()], outs=[z_full[t].ap().opt()])

        stage = getattr(cfg, "stage", 2)
        do_layer(0, 0)
        hook = None
        if stage >= 1:
            hook = lambda g: do_ag(0) if g == 4 else None
        do_layer(0, 1, mid_hook=hook)
        if stage >= 1:
            do_ag(1)
        if stage >= 2:
            do_layer(1, 0)
            do_layer(1, 1)

        for p in (pfin, ptrf, pagg, zpool, apool, mpool, ipool, gpool, cpool):
            p.release()

    nc.compile()
    return nc


def _make_inputs(cfg, meta, per_core, x_drug, x_dis, Ws, bs, lin_w, lin_b):
    bf = ml_dtypes.bfloat16
    perms = meta["perms"]
    xs = {0: np.asarray(x_drug), 1: np.asarray(x_dis)}
    xb = {}
    for sd in (0, 1):
        xp = np.empty_like(xs[sd])
        xp[perms[sd]] = xs[sd]      # row slot s holds node with perm[node]==s
        xb[sd] = xp.astype(bf)
    wts = np.zeros((2, 2, 2, 128, 128), np.float32)
    b1 = np.zeros((2, 128, 256), np.float32)
    b2 = np.zeros((2, 128, 1), np.float32)
    for l in (0, 1):
        for t in (0, 1):
            for ri in (0, 1):
                r = REL_OF_T[t][ri][0]
                wts[l, t, ri] = Ws[l, r]
            bsum = bs[l, REL_OF_T[t][0][0]] + bs[l, REL_OF_T[t][1][0]]
            if l == 0:
                b1[t] = np.tile(bsum[None, :], (128, 2))
            else:
                b2[t] = bsum[:, None].astype(np.float32)
    shared = {
        "wts": wts.astype(bf),
        "linwt": lin_w.T.astype(bf).copy(),
        "bias1": b1,
        "bias2": b2,
        "linb": np.tile(lin_b[None, :], (128, 2)).astype(np.float32),
    }
    for sd, nm in ((0, "d"), (1, "s")):
        for h in (0, 1):
            shared[f"x_{nm}_h{h}"] = np.ascontiguousarray(
                xb[sd][h * cfg.HALF:(h + 1) * cfg.HALF])
    in_maps = []
    for c in range(cfg.NC):
        m = dict(shared)
        m["idx"] = per_core[c]["idx"]
        m["msel"] = per_core[c]["msel"]
        in_maps.append(m)
    return in_maps


def run(cfg, x_drug, x_dis, eis, Ws, bs, lin_w, lin_b, trace=False):
    edge_arrays = {r: (eis[r][0].astype(np.int64), eis[r][1].astype(np.int64))
                   for r in range(4)}
    meta, per_core = _prep_graph(cfg, edge_arrays)
    nc = _build_program(cfg, meta)
    in_maps = _make_inputs(cfg, meta, per_core, x_drug, x_dis, Ws, bs,
                           lin_w, lin_b)
    res = run_bass_kernel_spmd(nc, in_maps, core_ids=list(range(cfg.NC)),
                               trace=trace)
    drug, dis = assemble(cfg, meta, [res.results[c]["out"]
                                     for c in range(cfg.NC)])
    return (drug, dis), res


def assemble(cfg, meta, outs):
    """outs[c] = per-core [2*S, OUT] slot-ordered output -> full node order."""
    slotted = {0: np.zeros((cfg.N, cfg.OUT), np.float32),
               1: np.zeros((cfg.N, cfg.OUT), np.float32)}
    for c in range(cfg.NC):
        o = outs[c]
        slotted[0][c * cfg.S:(c + 1) * cfg.S] = o[:cfg.S]
        slotted[1][c * cfg.S:(c + 1) * cfg.S] = o[cfg.S:]
    return (slotted[0][meta["perms"][0]], slotted[1][meta["perms"][1]])


def kernel(x_drug, x_dis, ei_dd, ei_ss, ei_ds, ei_sd, Ws, bs, lin_w, lin_b):
    cfg = Cfg()
    eis = {0: np.asarray(ei_dd), 1: np.asarray(ei_ss),
           2: np.asarray(ei_ds), 3: np.asarray(ei_sd)}
    out, _ = run(cfg, np.asarray(x_drug), np.asarray(x_dis), eis,
                 np.asarray(Ws), np.asarray(bs),
                 np.asarray(lin_w), np.asarray(lin_b))
    return out

